# revision 9
# baseline (speedup 1.0000x reference)
"""Trainium2 Bass kernel for nn_MultiHeadAttention_7834020348049.

Reference computation (per token, no cross-token interaction):
    qn  = LayerNorm(q) * gamma_m + beta_m
    kvn = LayerNorm(kv) * gamma_l + beta_l
    Q = qn @ Wq.T ; K,V = split(kvn @ Wkv.T)
    per token: scores[h,g] = Q[h,:] . K[g,:] / sqrt(128)  (8x8 over heads)
    ctx[h,:] = softmax_g(scores) @ V
    out = ctx @ Wo.T

Sharding: pure data-parallel over the 16*2048 = 32768 tokens -> 4096/core.

Per-core pipeline (feature-major, fp32r matmuls):
  token-major LN (bn_stats/bn_aggr + tensor_scalar)
  -> PE transpose to feature-major qn^T / kvn^T
  -> projections with weights stationary (fp32r, N=256)
  -> per 16-token sub-tile: scores matmul S[(t,h),(t',g)] (128x128),
     exp on ACT, block-diag-masked tensor_tensor_reduce for softmax sums,
     P = E*mask/Z, PE transpose of P -> block-diag L, context matmul
     ctx^T = Vb^T-style (Vb = PE-transposed V slice), token-major O-proj.
"""
import sys, os
sys.path.insert(0, "/opt/trn_rl_repo")
os.environ.setdefault("JAX_PLATFORMS", "cpu")

from contextlib import ExitStack
import numpy as np

import concourse.bass as bass
import concourse.bacc as bacc
import concourse.tile as tile
from concourse import mybir
from concourse.masks import make_identity
from concourse.bass_utils import run_bass_kernel_spmd

F32 = mybir.dt.float32
F32R = mybir.dt.float32r

DIM = 1024
HEADS = 8
DHEAD = 128
NCORES = 8

# tokens per chunk (projection moving-dim; must be >=256 for fp32r full rate)
TC = 256
# tokens per tile (partition dim)
TT = 128
# tokens per attention sub-tile
TS = 16


def R(ap):
    return ap.bitcast(F32R)


def build_nc(T, with_bias_q=False, with_bias_kv=False):
    """Build the single-core Bass program for T tokens."""
    nc = bacc.Bacc(trn_type="TRN2", target_bir_lowering=False)

    q_d = nc.dram_tensor("q", [T, DIM], F32, kind="ExternalInput").ap()
    kv_d = nc.dram_tensor("kv", [T, DIM], F32, kind="ExternalInput").ap()
    wq_d = nc.dram_tensor("wq", [DIM, DIM], F32, kind="ExternalInput").ap()
    wkv_d = nc.dram_tensor("wkv", [DIM, 2 * DIM], F32, kind="ExternalInput").ap()
    wo_d = nc.dram_tensor("wo", [DIM, DIM], F32, kind="ExternalInput").ap()
    mask_d = nc.dram_tensor("mask", [TT, TT], F32, kind="ExternalInput").ap()
    bq_d = bkv_d = None
    if with_bias_q:
        bq_d = nc.dram_tensor("bq", [1, DIM], F32, kind="ExternalInput").ap()
    if with_bias_kv:
        bkv_d = nc.dram_tensor("bkv", [1, 2 * DIM], F32, kind="ExternalInput").ap()
    out_d = nc.dram_tensor("out", [T, DIM], F32, kind="ExternalOutput").ap()

    KT_F = DIM // TT          # 8 k-tiles for the 1024-feature contraction
    NCH = T // TC             # chunks
    TPC = TC // TT            # tiles per chunk (2)
    SPT = TT // TS            # sub-tiles per tile (8)

    with tile.TileContext(nc) as tc, ExitStack() as ctx:
        # ---------------- static SBUF ----------------
        singles = ctx.enter_context(tc.tile_pool(name="singles", bufs=1))
        ident = singles.tile([128, 128], F32)
        make_identity(nc, ident[:])
        mask = singles.tile([TT, TT], F32)
        nc.sync.dma_start(mask[:], mask_d)

        wq_sb = singles.tile([128, KT_F, DIM], F32)
        wkv_sb = singles.tile([128, KT_F, 2 * DIM], F32)
        wo_sb = singles.tile([128, KT_F, DIM], F32)
        for k in range(KT_F):
            nc.sync.dma_start(R(wq_sb[:, k, :]), R(wq_d[k * 128:(k + 1) * 128, :]))
            nc.sync.dma_start(R(wkv_sb[:, k, :]), R(wkv_d[k * 128:(k + 1) * 128, :]))
            nc.sync.dma_start(R(wo_sb[:, k, :]), R(wo_d[k * 128:(k + 1) * 128, :]))
        if with_bias_q:
            bq_sb = singles.tile([1, DIM], F32)
            nc.sync.dma_start(R(bq_sb[:]), R(bq_d))
            ones_row = singles.tile([1, TC], F32)
            nc.vector.memset(R(ones_row[:]), 1.0)
        if with_bias_kv:
            bkv_sb = singles.tile([1, 2 * DIM], F32)
            nc.sync.dma_start(R(bkv_sb[:]), R(bkv_d))
            if not with_bias_q:
                ones_row = singles.tile([1, TC], F32)
                nc.vector.memset(R(ones_row[:]), 1.0)

        # chunk-level feature-major activations (single-buffered)
        chunk_sb = ctx.enter_context(tc.tile_pool(name="chunk", bufs=1))
        qnT = chunk_sb.tile([128, KT_F, TC], F32, tag="qnT")
        kvnT = chunk_sb.tile([128, KT_F, TC], F32, tag="kvnT")
        # interleaved (t, h) column layout: col = t*HEADS + h
        QT = chunk_sb.tile([128, TC * HEADS], F32, tag="QT")
        KT = chunk_sb.tile([128, TC * HEADS], F32, tag="KT")
        VT = chunk_sb.tile([128, TC * HEADS], F32, tag="VT")

        # rotating pools
        raw_p = ctx.enter_context(tc.tile_pool(name="raw", bufs=2))
        st_p = ctx.enter_context(tc.tile_pool(name="stats", bufs=3))
        sm_p = ctx.enter_context(tc.tile_pool(name="smax", bufs=2))
        l_p = ctx.enter_context(tc.tile_pool(name="lbuf", bufs=2))
        vb_p = ctx.enter_context(tc.tile_pool(name="vbuf", bufs=2))
        ctxT_p = ctx.enter_context(tc.tile_pool(name="ctxT", bufs=1))
        outsb_p = ctx.enter_context(tc.tile_pool(name="outsb", bufs=2))

        ps_mm = ctx.enter_context(tc.tile_pool(name="ps_mm", bufs=2, space="PSUM"))
        ps_tr = ctx.enter_context(tc.tile_pool(name="ps_tr", bufs=2, space="PSUM"))
        ps_s = ctx.enter_context(tc.tile_pool(name="ps_s", bufs=2, space="PSUM"))
        ps_pt = ctx.enter_context(tc.tile_pool(name="ps_pt", bufs=1, space="PSUM"))
        ps_vc = ctx.enter_context(tc.tile_pool(name="ps_vc", bufs=1, space="PSUM"))

        def layernorm_tile(x):
            """in-place LN over free dim (1024) of x [128, 1024]."""
            stats = st_p.tile([128, 2, 6], F32, tag="bn")
            xg = x.rearrange("p (n f) -> p n f", n=2)
            for i in range(2):
                nc.vector.bn_stats(out=stats[:, i, :], in_=xg[:, i, :])
            mv = st_p.tile([128, 2], F32, tag="mv")
            nc.vector.bn_aggr(out=mv[:], in_=stats[:])
            eps = st_p.tile([128, 1], F32, tag="eps")
            nc.vector.memset(eps[:], 1e-5)
            rstd = st_p.tile([128, 1], F32, tag="rstd")
            nc.scalar.activation(out=rstd[:], in_=mv[:, 1:2],
                                 func=mybir.ActivationFunctionType.Sqrt,
                                 bias=eps[:], scale=1.0)
            nc.vector.reciprocal(out=rstd[:], in_=rstd[:])
            nc.vector.tensor_scalar(out=x, in0=x,
                                    scalar1=mv[:, 0:1],
                                    scalar2=rstd[:],
                                    op0=mybir.AluOpType.subtract,
                                    op1=mybir.AluOpType.mult)

        def head_cols(tens, m):
            """stride-HEADS columns of head m in an interleaved tensor."""
            return bass.AP(tensor=tens.tensor, offset=tens.offset + m,
                           ap=[tens.ap[0], [HEADS, TC]])

        for c in range(NCH):
            # ---------- stage A: load + LN + transpose to feature-major ----
            for it in range(TPC):
                tok0 = c * TC + it * TT
                for name, src, dstT in (("q", q_d, qnT), ("kv", kv_d, kvnT)):
                    x = raw_p.tile([128, DIM], F32, tag="raw")
                    nc.sync.dma_start(x[:], src[tok0:tok0 + TT, :])
                    layernorm_tile(x[:])
                    for f in range(KT_F):
                        tp = ps_tr.tile([128, 128], F32)
                        nc.tensor.transpose(
                            tp[:], x[:, f * 128:(f + 1) * 128], ident[:])
                        nc.scalar.copy(
                            out=R(dstT[:, f, it * TT:(it + 1) * TT]), in_=tp[:])

            # ---------- stage B: Q / KV projections (feature-major) --------
            for m in range(HEADS):
                ps = ps_mm.tile([128, TC], F32, tag="mm")
                for k in range(KT_F):
                    nc.tensor.matmul(
                        ps[:], R(wq_sb[:, k, m * 128:(m + 1) * 128]),
                        R(qnT[:, k, :]), start=(k == 0),
                        stop=(k == KT_F - 1 and not with_bias_q))
                if with_bias_q:
                    nc.tensor.matmul(
                        ps[:], R(bq_sb[:, m * 128:(m + 1) * 128]),
                        R(ones_row[:]), start=False, stop=True)
                nc.scalar.copy(out=head_cols(QT, m), in_=ps[:])
            for m in range(2 * HEADS):
                ps = ps_mm.tile([128, TC], F32, tag="mm")
                for k in range(KT_F):
                    nc.tensor.matmul(
                        ps[:], R(wkv_sb[:, k, m * 128:(m + 1) * 128]),
                        R(kvnT[:, k, :]), start=(k == 0),
                        stop=(k == KT_F - 1 and not with_bias_kv))
                if with_bias_kv:
                    nc.tensor.matmul(
                        ps[:], R(bkv_sb[:, m * 128:(m + 1) * 128]),
                        R(ones_row[:]), start=False, stop=True)
                dst = KT if m < HEADS else VT
                nc.scalar.copy(out=head_cols(dst, m % HEADS), in_=ps[:])

            # ---------- stage C: attention + O-projection per tile ---------
            for it in range(TPC):
                tok0 = c * TC + it * TT
                ctxT = ctxT_p.tile([128, HEADS, TT], F32, tag="ctxT")
                for s in range(SPT):
                    c0 = (it * TT + s * TS) * HEADS   # interleaved col base
                    # scores S[(t,h), (t',g)]
                    sps = ps_s.tile([128, 128], F32)
                    nc.tensor.matmul(sps[:], QT[:, c0:c0 + 128],
                                     KT[:, c0:c0 + 128],
                                     start=True, stop=True)
                    # E = exp(S)  (junk cross-token entries included, masked next)
                    e = sm_p.tile([128, 128], F32, tag="e")
                    nc.scalar.activation(out=e[:], in_=sps[:],
                                         func=mybir.ActivationFunctionType.Exp,
                                         scale=1.0)
                    # EM = E * mask ; Z = rowsum(EM)
                    em = sm_p.tile([128, 128], F32, tag="em")
                    z = st_p.tile([128, 1], F32, tag="z")
                    nc.vector.tensor_tensor(out=em[:], in0=e[:], in1=mask[:],
                                            op=mybir.AluOpType.mult)
                    nc.vector.tensor_reduce(out=z[:], in_=em[:],
                                            op=mybir.AluOpType.add,
                                            axis=mybir.AxisListType.X)
                    zr = st_p.tile([128, 1], F32, tag="zr")
                    nc.vector.reciprocal(out=zr[:], in_=z[:])
                    # P = EM / Z
                    p = sm_p.tile([128, 128], F32, tag="p")
                    nc.vector.tensor_scalar(out=p[:], in0=em[:],
                                            scalar1=zr[:], scalar2=None,
                                            op0=mybir.AluOpType.mult)
                    # L = P^T  (block-diagonal by construction)
                    ptp = ps_pt.tile([128, 128], F32)
                    nc.tensor.transpose(ptp[:], p[:], ident[:])
                    lbuf = l_p.tile([128, 128], F32, tag="l")
                    nc.vector.tensor_copy(out=lbuf[:], in_=ptp[:])
                    # Vb[(t,g), d] = transpose of V^T token-columns
                    vbp = ps_vc.tile([128, 128], F32, tag="vc")
                    nc.tensor.transpose(vbp[:], VT[:, c0:c0 + 128],
                                        ident[:])
                    vb = vb_p.tile([128, 128], F32, tag="vb")
                    nc.vector.tensor_copy(out=vb[:], in_=vbp[:])
                    # ctx^T[d, (t,h)] = Vb^T @ L
                    cps = ps_vc.tile([128, 128], F32, tag="vc")
                    nc.tensor.matmul(cps[:], vb[:], lbuf[:],
                                     start=True, stop=True)
                    ctx_dst = bass.AP(
                        tensor=ctxT.tensor,
                        offset=ctxT.offset + s * TS,
                        ap=[ctxT.ap[0], [1, TS], [TT, HEADS]])
                    nc.scalar.copy(out=R(ctx_dst), in_=cps[:])

                # O-projection, token-major: out[t, o] += ctxT_h^T @ wo_h
                for nn2 in range(2):
                    pso = ps_mm.tile([128, 512], F32, tag="mm")
                    for h in range(HEADS):
                        nc.tensor.matmul(
                            pso[:], R(ctxT[:, h, :]),
                            R(wo_sb[:, h, nn2 * 512:(nn2 + 1) * 512]),
                            start=(h == 0), stop=(h == HEADS - 1))
                    osb = outsb_p.tile([128, 512], F32, tag="osb")
                    nc.scalar.copy(out=osb[:], in_=pso[:])
                    nc.sync.dma_start(
                        out_d[tok0:tok0 + TT, nn2 * 512:(nn2 + 1) * 512], osb[:])

    nc.finalize()
    return nc


def _host_mask():
    m = np.zeros((TT, TT), np.float32)
    p = np.arange(TT)
    m[p[:, None] // HEADS == p[None, :] // HEADS] = 1.0
    return m


def kernel(q, kv, gamma_m, beta_m, gamma_l, beta_l, Wq, Wkv, Wo):
    q = np.asarray(q, np.float32)
    kv = np.asarray(kv, np.float32)
    bs, patch, _ = q.shape
    T_total = bs * patch
    T_core = T_total // NCORES

    scale = DHEAD ** (-0.5)
    # fold LN gamma into the projection weights, beta into bias vectors
    wq_eff = (np.asarray(Wq, np.float32) * np.asarray(gamma_m, np.float32)[None, :]) * scale
    bq = (np.asarray(Wq, np.float32) @ np.asarray(beta_m, np.float32)) * scale
    wkv_eff = np.asarray(Wkv, np.float32) * np.asarray(gamma_l, np.float32)[None, :]
    bkv = np.asarray(Wkv, np.float32) @ np.asarray(beta_l, np.float32)
    with_bias_q = bool(np.any(bq != 0.0))
    with_bias_kv = bool(np.any(bkv != 0.0))

    # kernel weight layout: [in, out]
    wq_t = np.ascontiguousarray(wq_eff.T)
    wkv_t = np.ascontiguousarray(wkv_eff.T)
    wo_t = np.ascontiguousarray(np.asarray(Wo, np.float32).T)
    mask = _host_mask()

    nc = build_nc(T_core, with_bias_q, with_bias_kv)

    qf = q.reshape(T_total, DIM)
    kvf = kv.reshape(T_total, DIM)
    in_maps = []
    for i in range(NCORES):
        m = {
            "q": np.ascontiguousarray(qf[i * T_core:(i + 1) * T_core]),
            "kv": np.ascontiguousarray(kvf[i * T_core:(i + 1) * T_core]),
            "wq": wq_t, "wkv": wkv_t, "wo": wo_t, "mask": mask,
        }
        if with_bias_q:
            m["bq"] = bq.reshape(1, DIM).astype(np.float32)
        if with_bias_kv:
            m["bkv"] = bkv.reshape(1, 2 * DIM).astype(np.float32)
        in_maps.append(m)

    res = run_bass_kernel_spmd(nc, in_maps, list(range(NCORES)))
    global LAST_RESULTS
    LAST_RESULTS = res
    out = np.concatenate([res.results[i]["out"] for i in range(NCORES)], axis=0)
    return out.reshape(bs, patch, DIM)


LAST_RESULTS = None
